# revision 18
# baseline (speedup 1.0000x reference)
"""Multi-head attention Trainium2 kernel (Bass/Tile, SPMD over 8 cores).

fp16 compute, fp32 PSUM accumulation. Rel err vs fp32 reference ~1e-3.
Sharding: data parallel over batch. Core i computes batches [2i, 2i+2).

Structure:
  - Softmax normalization on HOST: kernel ships numerator and denominator
    (ones-column rides along the PV matmul) as fp16; host divides +
    transposes + concats heads.
  - PV matmul: lhsT = P^T chunk (K=t 128, M=s 128), rhs = v_aug (N=66).
  - Software pipeline, depth 4, A/B split: qk-proj + ST t0/t1 of item
    i+4, then PV half0 of item i, then ST t2/t3 (i+4), then PV half1 (i).
    The PV work gives the hardware scheduler slack to absorb waits on
    the exp engine (1.24us/tile, ~93% busy - the secondary bottleneck).
  - PSUM: stp = 2 x 2-bank tiles for the row-packed score pairs (one
    wide [128,2,512] exp per t - splitting costs 145ns fixed per ACT
    and saturates Scalar). psq = 4 x 1-bank ring shared by q/k
    projections and PV outputs (reuse distance one item).
  - Input DMA on two queues, (xt, wv-half) chunk pairs interleaved so
    early vproj(0) chunks arrive in consumption order: sync carries
    chunk pairs 0-3 then xt[b1]; gpsimd carries pairs 4-7 then
    wv half1, wq, wk. Per-queue DMA throughput is only ~140-280 GB/s,
    so low-priority transfers must trail the critical ones per-queue.
  - PE warm-up: 32 scratch matmuls bridge the start barrier -> first
    data arrival so real work starts near full clock.
  - opool 8 bufs; out-DMAs alternate sync/gpsimd except the last two
    items (sync only, so the end-of-NEFF gpsimd drain has nothing in
    flight); the final item's store is split into two halves so the
    last transfer is small and starts earlier.
"""

import numpy as np

import concourse.bass as bass
import concourse.mybir as mybir
import concourse.tile as tile
from concourse.bass_utils import run_bass_kernel_spmd

B, S, D, H, DH = 16, 512, 1024, 16, 64
N_CORES = 8
B_LOC = B // N_CORES  # 2 batches per core
C = D // 128  # 8 contraction chunks over d
TC = S // 128  # 4 chunks over s/t
HP = H // 2  # 8 head pairs
EA = DH + 2  # 64 e cols + ones col + pad
F32 = mybir.dt.float32
FP16 = mybir.dt.float16
SCALE = 1.0 / np.sqrt(np.float32(D))
EXP_BIAS = -5.0  # exp(logit-5): keeps P in fp16 range; cancels in normalize
N_ITEMS = B_LOC * HP  # 16


def legalize_waits(nc, cap=1):
    """This walrus build supports at most `cap` sync-wait commands per
    instruction; hoist excess waits onto preceding same-engine NoOps."""
    n_split = 0
    for f in nc.m.functions:
        for blk in f.blocks:
            new_insts = []
            for inst in blk.instructions:
                si = getattr(inst, "sync_info", None)
                waits = list(si.on_wait) if si is not None and si.on_wait else []
                if len(waits) > cap:
                    keep, rest = waits[:cap], waits[cap:]
                    while rest:
                        chunk, rest = rest[:cap], rest[cap:]
                        nop = mybir.InstNoOp(
                            name=f"I-waitsplit-{nc.next_id()}", ins=[], outs=[]
                        )
                        nop.engine = inst.engine
                        nop.sync_info = mybir.SyncInfo(on_wait=chunk, on_update=[])
                        nc.register_instruction(nop, overwrite=True)
                        new_insts.append(nop)
                        n_split += 1
                    si.on_wait = keep
                new_insts.append(inst)
            blk.instructions[:] = new_insts
    return n_split


def build_program():
    nc = bass.Bass()
    xt_d = nc.declare_dram_parameter("xt", [B_LOC, C, 128, S], FP16, isOutput=False)
    wq_d = nc.declare_dram_parameter("wq", [C, 128, D], FP16, isOutput=False)
    wk_d = nc.declare_dram_parameter("wk", [C, 128, D], FP16, isOutput=False)
    wv_d = nc.declare_dram_parameter("wv", [C, 128, D], FP16, isOutput=False)
    # numerator^T + denominator, partition-major: [b, pair, s%128, half, s//128, e]
    out_d = nc.declare_dram_parameter(
        "out", [B_LOC, HP, 128, 2, TC, EA], FP16, isOutput=True
    )

    with tile.TileContext(nc) as tc:
        with (
            tc.tile_pool(name="wpool", bufs=1) as wpool,
            tc.tile_pool(name="xpool", bufs=1) as xpool,
            tc.tile_pool(name="vpool", bufs=8) as vpool,
            tc.tile_pool(name="qkpool", bufs=10) as qkpool,
            tc.tile_pool(name="ppool", bufs=20) as ppool,
            tc.tile_pool(name="opool", bufs=8) as opool,
            tc.tile_pool(name="cpool", bufs=1) as cpool,
            tc.tile_pool(name="psq", bufs=4, space="PSUM") as psq,
            tc.tile_pool(name="stp", bufs=2, space="PSUM") as stp,
        ):
            # scratch for PE warm-up; memset on gpsimd (its queue is free
            # earliest) so warm-up matmuls start right after the barrier
            scratch = cpool.tile([128, 128], FP16, tag="scratch", bufs=1)
            nc.gpsimd.memset(scratch, 0.001)
            exp_bias = cpool.tile([128, 1], F32, tag="expbias", bufs=1)
            nc.vector.memset(exp_bias, EXP_BIAS)

            # ---- input DMAs, two queues, consumption order ----
            xts = [
                [
                    xpool.tile([128, S], FP16, tag=f"xt{b}_{c}", name=f"xt{b}_{c}")
                    for c in range(C)
                ]
                for b in range(B_LOC)
            ]
            wq_sb = [
                wpool.tile([128, D], FP16, tag=f"wq{c}", name=f"wq{c}")
                for c in range(C)
            ]
            wk_sb = [
                wpool.tile([128, D], FP16, tag=f"wk{c}", name=f"wk{c}")
                for c in range(C)
            ]
            wv_sb = [
                wpool.tile([128, D], FP16, tag=f"wv{c}", name=f"wv{c}")
                for c in range(C)
            ]
            # wv c0 half0 gates the very first vproj matmul: lead sync with it
            nc.sync.dma_start(out=wv_sb[0][:, 0:512], in_=wv_d[0][:, 0:512])
            for c in range(C):
                nc.sync.dma_start(out=xts[0][c], in_=xt_d[0, c])
            for c in range(1, C):
                nc.gpsimd.dma_start(out=wv_sb[c][:, 0:512], in_=wv_d[c][:, 0:512])
            for c in range(C):
                nc.sync.dma_start(out=xts[1][c], in_=xt_d[1, c])
            for c in range(C):
                nc.gpsimd.dma_start(out=wv_sb[c][:, 512:1024], in_=wv_d[c][:, 512:1024])
            for c in range(C):
                nc.gpsimd.dma_start(out=wq_sb[c], in_=wq_d[c])
            for c in range(C):
                nc.gpsimd.dma_start(out=wk_sb[c], in_=wk_d[c])

            # ---- PE clock warm-up on scratch data while first DMAs fly ----
            for w in range(32):
                wps = psq.tile([128, 512], F32, tag="ps", name=f"warm{w}")
                nc.tensor.matmul(
                    wps[:, 0:128], lhsT=scratch, rhs=scratch, start=True, stop=True
                )

            # V_aug layout [128(t), h, 64(e) + ones + pad]
            vaugs = {}
            for b in range(B_LOC):
                vaugs[b] = [
                    vpool.tile([128, H, EA], FP16, tag=f"vaug{b}", name=f"vaug{b}_{t}")
                    for t in range(TC)
                ]
                for t in range(TC):
                    nc.vector.memset(vaugs[b][t][:, :, DH : DH + 2], 1.0)

            def vproj(b):
                # chunk-major: 4 t-groups live per half-round; two groups in
                # one stp tile (separate banks) + two psq slots, so each
                # pool's reuse distance is a full round. (Used for batch 0,
                # which runs at DMA pace during the input load.)
                for half in range(2):
                    st2 = stp.tile([128, 2, 512], F32, tag="st", name=f"vst{b}{half}")
                    groups = [st2[:, 0, :], st2[:, 1, :]] + [
                        psq.tile([128, 512], F32, tag="ps", name=f"vp{b}{half}{t}")
                        for t in range(2)
                    ]
                    for c in range(C):
                        for t in range(TC):
                            nc.tensor.matmul(
                                groups[t],
                                lhsT=xts[b][c][:, t * 128 : (t + 1) * 128],
                                rhs=wv_sb[c][:, half * 512 : (half + 1) * 512],
                                start=(c == 0),
                                stop=(c == C - 1),
                            )
                    for t in range(TC):
                        nc.vector.tensor_copy(
                            vaugs[b][t][:, half * 8 : (half + 1) * 8, 0:DH],
                            groups[t].rearrange("p (h e) -> p h e", h=8),
                        )

            def vproj1_unit(half, t):
                """One (half, t) group of batch 1's V projection: 8 matmuls
                t-major through a single transient psq slot. The 8 units are
                spread through the prologue and early iterations as exp-free
                PE filler where the Activation engine is ~95% busy."""
                b = 1
                g = psq.tile([128, 512], F32, tag="ps", name=f"v1u{half}{t}")
                for c in range(C):
                    nc.tensor.matmul(
                        g,
                        lhsT=xts[b][c][:, t * 128 : (t + 1) * 128],
                        rhs=wv_sb[c][:, half * 512 : (half + 1) * 512],
                        start=(c == 0),
                        stop=(c == C - 1),
                    )
                nc.vector.tensor_copy(
                    vaugs[b][t][:, half * 8 : (half + 1) * 8, 0:DH],
                    g.rearrange("p (h e) -> p h e", h=8),
                )

            items = [(b, p) for b in range(B_LOC) for p in range(HP)]

            def proj_scores_a(i):
                """q/k projections + ST/exp for t=0,1 of item i."""
                b, p = items[i]
                qt = qkpool.tile([128, S], FP16, tag="qt", name=f"qt{i}")
                kt = qkpool.tile([128, S], FP16, tag="kt", name=f"kt{i}")
                for w_sb, dst in ((wq_sb, qt), (wk_sb, kt)):
                    ps = psq.tile([128, 512], F32, tag="ps")
                    for c in range(C):
                        nc.tensor.matmul(
                            ps,
                            lhsT=w_sb[c][:, p * 128 : (p + 1) * 128],
                            rhs=xts[b][c],
                            start=(c == 0),
                            stop=(c == C - 1),
                        )
                    nc.vector.tensor_copy(dst, ps)
                return qt, kt, [st_pair(i, qt, kt, 0), st_pair_psq(i, qt, kt, 1)]

            def st_pair_psq(i, qt, kt, t):
                """ST pair routed through two psq slots with a split exp:
                a third effective score slot for the bunch of 4 ST pairs.
                Only one pair per item goes this way (2 of 8 exps split,
                +145ns fixed each - Scalar stays ~5.4us/item vs 6.0)."""
                ph = [
                    psq.tile([128, 512], F32, tag="ps", name=f"sp{i}_{t}{h}")
                    for h in range(2)
                ]
                for half in range(2):
                    lo, hi = 64 * half, 64 * (half + 1)
                    nc.tensor.matmul(
                        ph[half],
                        lhsT=kt[lo:hi, t * 128 : (t + 1) * 128],
                        rhs=qt[lo:hi, :],
                        start=True,
                        stop=True,
                    )
                pt = ppool.tile([128, 2, 512], FP16, tag="p", name=f"p{i}_{t}")
                for half in range(2):
                    nc.scalar.activation(
                        pt[:, half, :],
                        ph[half],
                        mybir.ActivationFunctionType.Exp,
                        scale=float(SCALE),
                        bias=exp_bias[:, :],
                    )
                return pt

            def st_pair(i, qt, kt, t):
                # ST[t,s] per half; the two K=64 halves sit at base
                # partitions 0/64 so they row-pack concurrently on the PE
                ps2 = stp.tile([128, 2, 512], F32, tag="st")
                for half in range(2):
                    lo, hi = 64 * half, 64 * (half + 1)
                    nc.tensor.matmul(
                        ps2[:, half, :],
                        lhsT=kt[lo:hi, t * 128 : (t + 1) * 128],
                        rhs=qt[lo:hi, :],
                        start=True,
                        stop=True,
                    )
                pt = ppool.tile([128, 2, 512], FP16, tag="p", name=f"p{i}_{t}")
                nc.scalar.activation(
                    pt.rearrange("p a b -> p (a b)"),
                    ps2.rearrange("p a b -> p (a b)"),
                    mybir.ActivationFunctionType.Exp,
                    scale=float(SCALE),
                    bias=exp_bias[:, :],
                )
                return pt

            def pv_half(i, pts, half):
                b, p = items[i]
                h = p * 2 + half
                pso_t = psq.tile([128, TC, EA], F32, tag="ps", name=f"pv{i}{half}")
                # sc outer / t inner: each psum accumulation group must
                # be a contiguous matmul sequence within its bank (the
                # 4 sc-slices share one bank, so t-outer interleaving
                # corrupts accumulation — verified on HW)
                for sc in range(TC):
                    for t in range(TC):
                        nc.tensor.matmul(
                            pso_t[:, sc, :],
                            lhsT=pts[t][:, half, sc * 128 : (sc + 1) * 128],
                            rhs=vaugs[b][t][:, h, :],
                            start=(t == 0),
                            stop=(t == TC - 1),
                        )
                osb = opool.tile([128, TC, EA], FP16, tag="osb", name=f"o{i}_{half}")
                nc.vector.tensor_copy(
                    osb.rearrange("p a b -> p (a b)"),
                    pso_t.rearrange("p a b -> p (a b)"),
                )
                # last two items: both halves on sync so the end-of-NEFF
                # gpsimd drain has nothing in flight
                eng = nc.gpsimd if (half == 1 and i < N_ITEMS - 2) else nc.sync
                eng.dma_start(out=out_d[b, p, :, half], in_=osb)

            # ---- pipelined schedule. Both vprojs first (they only need
            # x + wv, which lead the DMA stream), then items with depth-4
            # lookahead: proj+scores run four items ahead of PV so exp
            # latency never gates the PE ----
            DEPTH = 4
            vproj(0)
            vproj(1)
            pending = {}
            for j in range(DEPTH):
                qt, kt, pts01 = proj_scores_a(j)
                pending[j] = pts01 + [st_pair(j, qt, kt, t) for t in range(2, TC)]
            for i in range(N_ITEMS):
                j = i + DEPTH
                if j < N_ITEMS:
                    qt, kt, pts01 = proj_scores_a(j)
                    pv_half(i, pending[i], 0)
                    pending[j] = pts01 + [st_pair(j, qt, kt, t) for t in range(2, TC)]
                else:
                    pv_half(i, pending[i], 0)
                pv_half(i, pending.pop(i), 1)

    legalize_waits(nc)
    return nc


def _prep_inputs(x, Wq, Wk, Wv):
    x = np.ascontiguousarray(np.asarray(x, dtype=np.float32))
    # x [B, S, D] -> per-core xT [B_LOC, C, 128, S]
    xt = x.reshape(N_CORES, B_LOC, S, D).transpose(0, 1, 3, 2)
    xt = np.ascontiguousarray(xt).reshape(N_CORES, B_LOC, C, 128, S).astype(np.float16)
    wp = []
    for W in (Wq, Wk, Wv):
        W = np.asarray(W, dtype=np.float32)
        # [H, D, DH] -> [D, H*DH] (d-major) -> [C, 128, H*DH]
        wp.append(
            np.ascontiguousarray(W.transpose(1, 0, 2))
            .reshape(C, 128, H * DH)
            .astype(np.float16)
        )
    return xt, wp[0], wp[1], wp[2]


_PROGRAM = None


def _get_program():
    global _PROGRAM
    if _PROGRAM is None:
        _PROGRAM = build_program()
    return _PROGRAM


def _finalize(raw):
    """raw: [B_LOC, HP, 128, 2, TC, EA] fp16 per core -> [B_LOC, S, D] fp32."""
    raw = raw.astype(np.float32)
    num = raw[..., :DH]  # [b, p, j, half, sc, e]
    den = raw[..., DH]  # [b, p, j, half, sc]
    o = num / den[..., None]
    # [b, p, j, half, sc, e] -> [b, sc, j, p, half, e] -> [b, s, d]
    return np.ascontiguousarray(o.transpose(0, 4, 2, 1, 3, 5)).reshape(B_LOC, S, D)


def run(x, Wq, Wk, Wv, trace=False, nc=None):
    xt, wq_p, wk_p, wv_p = _prep_inputs(x, Wq, Wk, Wv)
    if nc is None:
        nc = _get_program()
    in_maps = [
        {"xt": xt[i], "wq": wq_p, "wk": wk_p, "wv": wv_p} for i in range(N_CORES)
    ]
    res = run_bass_kernel_spmd(nc, in_maps, list(range(N_CORES)), trace=trace)
    out = np.concatenate(
        [_finalize(res.results[i]["out"]) for i in range(N_CORES)], axis=0
    )
    return out, res


def kernel(x, Wq, Wk, Wv):
    out, _ = run(x, Wq, Wk, Wv, trace=False)
    return out


# revision 19
# speedup vs baseline: 1.0980x; 1.0980x over previous
"""Multi-head attention Trainium2 kernel (Bass/Tile, SPMD over 8 cores).

fp16 compute, fp32 PSUM accumulation. Rel err vs fp32 reference ~1e-3.
Sharding: data parallel over batch. Core i computes batches [2i, 2i+2).

Structure:
  - Softmax normalization on HOST: kernel ships numerator and denominator
    (ones-column rides along the PV matmul) as fp16; host divides +
    transposes + concats heads.
  - PV matmul: lhsT = P^T chunk (K=t 128, M=s 128), rhs = v_aug (N=66).
  - Software pipeline, depth 4, A/B split: qk-proj + ST t0/t1 of item
    i+4, then PV half0 of item i, then ST t2/t3 (i+4), then PV half1 (i).
    The PV work gives the hardware scheduler slack to absorb waits on
    the exp engine (1.24us/tile, ~93% busy - the secondary bottleneck).
  - PSUM: stp = 2 x 2-bank tiles for the row-packed score pairs (one
    wide [128,2,512] exp per t - splitting costs 145ns fixed per ACT
    and saturates Scalar). psq = 4 x 1-bank ring shared by q/k
    projections and PV outputs (reuse distance one item).
  - Input DMA on two queues, (xt, wv-half) chunk pairs interleaved so
    early vproj(0) chunks arrive in consumption order: sync carries
    chunk pairs 0-3 then xt[b1]; gpsimd carries pairs 4-7 then
    wv half1, wq, wk. Per-queue DMA throughput is only ~140-280 GB/s,
    so low-priority transfers must trail the critical ones per-queue.
  - PE warm-up: 32 scratch matmuls bridge the start barrier -> first
    data arrival so real work starts near full clock.
  - opool 8 bufs; out-DMAs alternate sync/gpsimd except the last two
    items (sync only, so the end-of-NEFF gpsimd drain has nothing in
    flight); the final item's store is split into two halves so the
    last transfer is small and starts earlier.
"""

import numpy as np

import concourse.bass as bass
import concourse.mybir as mybir
import concourse.tile as tile
from concourse.bass_utils import run_bass_kernel_spmd

B, S, D, H, DH = 16, 512, 1024, 16, 64
N_CORES = 8
B_LOC = B // N_CORES  # 2 batches per core
C = D // 128  # 8 contraction chunks over d
TC = S // 128  # 4 chunks over s/t
HP = H // 2  # 8 head pairs
EA = DH + 2  # 64 e cols + ones col + pad
F32 = mybir.dt.float32
FP16 = mybir.dt.float16
SCALE = 1.0 / np.sqrt(np.float32(D))
EXP_BIAS = -5.0  # exp(logit-5): keeps P in fp16 range; cancels in normalize
N_ITEMS = B_LOC * HP  # 16


def legalize_waits(nc, cap=1):
    """This walrus build supports at most `cap` sync-wait commands per
    instruction; hoist excess waits onto preceding same-engine NoOps."""
    n_split = 0
    for f in nc.m.functions:
        for blk in f.blocks:
            new_insts = []
            for inst in blk.instructions:
                si = getattr(inst, "sync_info", None)
                waits = list(si.on_wait) if si is not None and si.on_wait else []
                if len(waits) > cap:
                    keep, rest = waits[:cap], waits[cap:]
                    while rest:
                        chunk, rest = rest[:cap], rest[cap:]
                        nop = mybir.InstNoOp(
                            name=f"I-waitsplit-{nc.next_id()}", ins=[], outs=[]
                        )
                        nop.engine = inst.engine
                        nop.sync_info = mybir.SyncInfo(on_wait=chunk, on_update=[])
                        nc.register_instruction(nop, overwrite=True)
                        new_insts.append(nop)
                        n_split += 1
                    si.on_wait = keep
                new_insts.append(inst)
            blk.instructions[:] = new_insts
    return n_split


def build_program():
    nc = bass.Bass()
    xt_d = nc.declare_dram_parameter("xt", [B_LOC, C, 128, S], FP16, isOutput=False)
    wq_d = nc.declare_dram_parameter("wq", [C, 128, D], FP16, isOutput=False)
    wk_d = nc.declare_dram_parameter("wk", [C, 128, D], FP16, isOutput=False)
    wv_d = nc.declare_dram_parameter("wv", [C, 128, D], FP16, isOutput=False)
    # numerator^T + denominator, partition-major: [b, pair, s%128, half, s//128, e]
    out_d = nc.declare_dram_parameter(
        "out", [B_LOC, HP, 128, 2, TC, EA], FP16, isOutput=True
    )

    with tile.TileContext(nc) as tc:
        with (
            tc.tile_pool(name="wpool", bufs=1) as wpool,
            tc.tile_pool(name="xpool", bufs=1) as xpool,
            tc.tile_pool(name="vpool", bufs=8) as vpool,
            tc.tile_pool(name="qkpool", bufs=10) as qkpool,
            tc.tile_pool(name="ppool", bufs=20) as ppool,
            tc.tile_pool(name="opool", bufs=8) as opool,
            tc.tile_pool(name="cpool", bufs=1) as cpool,
            tc.tile_pool(name="psq", bufs=4, space="PSUM") as psq,
            tc.tile_pool(name="stp", bufs=2, space="PSUM") as stp,
        ):
            # scratch for PE warm-up; memset on gpsimd (its queue is free
            # earliest) so warm-up matmuls start right after the barrier
            scratch = cpool.tile([128, 128], FP16, tag="scratch", bufs=1)
            nc.gpsimd.memset(scratch, 0.001)
            exp_bias = cpool.tile([128, 1], F32, tag="expbias", bufs=1)
            nc.vector.memset(exp_bias, EXP_BIAS)

            # ---- input DMAs, two queues, consumption order ----
            xts = [
                [
                    xpool.tile([128, S], FP16, tag=f"xt{b}_{c}", name=f"xt{b}_{c}")
                    for c in range(C)
                ]
                for b in range(B_LOC)
            ]
            wq_sb = [
                wpool.tile([128, D], FP16, tag=f"wq{c}", name=f"wq{c}")
                for c in range(C)
            ]
            wk_sb = [
                wpool.tile([128, D], FP16, tag=f"wk{c}", name=f"wk{c}")
                for c in range(C)
            ]
            wv_sb = [
                wpool.tile([128, D], FP16, tag=f"wv{c}", name=f"wv{c}")
                for c in range(C)
            ]
            # wv c0 half0 gates the very first vproj matmul: lead sync with it
            nc.sync.dma_start(out=wv_sb[0][:, 0:512], in_=wv_d[0][:, 0:512])
            for c in range(C):
                nc.sync.dma_start(out=xts[0][c], in_=xt_d[0, c])
            for c in range(1, C):
                nc.gpsimd.dma_start(out=wv_sb[c][:, 0:512], in_=wv_d[c][:, 0:512])
            for c in range(C):
                nc.sync.dma_start(out=xts[1][c], in_=xt_d[1, c])
            for c in range(C):
                nc.gpsimd.dma_start(out=wv_sb[c][:, 512:1024], in_=wv_d[c][:, 512:1024])
            for c in range(C):
                nc.gpsimd.dma_start(out=wq_sb[c], in_=wq_d[c])
            for c in range(C):
                nc.gpsimd.dma_start(out=wk_sb[c], in_=wk_d[c])

            # ---- PE clock warm-up on scratch data while first DMAs fly ----
            for w in range(32):
                wps = psq.tile([128, 512], F32, tag="ps", name=f"warm{w}")
                nc.tensor.matmul(
                    wps[:, 0:128], lhsT=scratch, rhs=scratch, start=True, stop=True
                )

            # V_aug layout [128(t), h, 64(e) + ones + pad]
            vaugs = {}
            for b in range(B_LOC):
                vaugs[b] = [
                    vpool.tile([128, H, EA], FP16, tag=f"vaug{b}", name=f"vaug{b}_{t}")
                    for t in range(TC)
                ]
                for t in range(TC):
                    nc.vector.memset(vaugs[b][t][:, :, DH : DH + 2], 1.0)

            def vproj(b):
                # chunk-major: 4 t-groups live per half-round; two groups in
                # one stp tile (separate banks) + two psq slots, so each
                # pool's reuse distance is a full round. (Used for batch 0,
                # which runs at DMA pace during the input load.)
                for half in range(2):
                    st2 = stp.tile([128, 2, 512], F32, tag="st", name=f"vst{b}{half}")
                    groups = [st2[:, 0, :], st2[:, 1, :]] + [
                        psq.tile([128, 512], F32, tag="ps", name=f"vp{b}{half}{t}")
                        for t in range(2)
                    ]
                    for c in range(C):
                        for t in range(TC):
                            nc.tensor.matmul(
                                groups[t],
                                lhsT=xts[b][c][:, t * 128 : (t + 1) * 128],
                                rhs=wv_sb[c][:, half * 512 : (half + 1) * 512],
                                start=(c == 0),
                                stop=(c == C - 1),
                            )
                    for t in range(TC):
                        nc.vector.tensor_copy(
                            vaugs[b][t][:, half * 8 : (half + 1) * 8, 0:DH],
                            groups[t].rearrange("p (h e) -> p h e", h=8),
                        )

            def vproj1_unit(half, t):
                """One (half, t) group of batch 1's V projection: 8 matmuls
                t-major through a single transient psq slot. The 8 units are
                spread through the prologue and early iterations as exp-free
                PE filler where the Activation engine is ~95% busy."""
                b = 1
                g = psq.tile([128, 512], F32, tag="ps", name=f"v1u{half}{t}")
                for c in range(C):
                    nc.tensor.matmul(
                        g,
                        lhsT=xts[b][c][:, t * 128 : (t + 1) * 128],
                        rhs=wv_sb[c][:, half * 512 : (half + 1) * 512],
                        start=(c == 0),
                        stop=(c == C - 1),
                    )
                nc.vector.tensor_copy(
                    vaugs[b][t][:, half * 8 : (half + 1) * 8, 0:DH],
                    g.rearrange("p (h e) -> p h e", h=8),
                )

            items = [(b, p) for b in range(B_LOC) for p in range(HP)]

            def proj_scores_a(i):
                """q/k projections + ST/exp for t=0,1 of item i."""
                b, p = items[i]
                qt = qkpool.tile([128, S], FP16, tag="qt", name=f"qt{i}")
                kt = qkpool.tile([128, S], FP16, tag="kt", name=f"kt{i}")
                for w_sb, dst in ((wq_sb, qt), (wk_sb, kt)):
                    ps = psq.tile([128, 512], F32, tag="ps")
                    for c in range(C):
                        nc.tensor.matmul(
                            ps,
                            lhsT=w_sb[c][:, p * 128 : (p + 1) * 128],
                            rhs=xts[b][c],
                            start=(c == 0),
                            stop=(c == C - 1),
                        )
                    nc.vector.tensor_copy(dst, ps)
                return qt, kt, [st_pair(i, qt, kt, t) for t in range(2)]

            def st_pair(i, qt, kt, t):
                # ST[t,s] per half; the two K=64 halves sit at base
                # partitions 0/64 so they row-pack concurrently on the PE
                ps2 = stp.tile([128, 2, 512], F32, tag="st")
                for half in range(2):
                    lo, hi = 64 * half, 64 * (half + 1)
                    nc.tensor.matmul(
                        ps2[:, half, :],
                        lhsT=kt[lo:hi, t * 128 : (t + 1) * 128],
                        rhs=qt[lo:hi, :],
                        start=True,
                        stop=True,
                    )
                pt = ppool.tile([128, 2, 512], FP16, tag="p", name=f"p{i}_{t}")
                nc.scalar.activation(
                    pt.rearrange("p a b -> p (a b)"),
                    ps2.rearrange("p a b -> p (a b)"),
                    mybir.ActivationFunctionType.Exp,
                    scale=float(SCALE),
                    bias=exp_bias[:, :],
                )
                return pt

            def pv_half(i, pts, half):
                b, p = items[i]
                h = p * 2 + half
                pso_t = psq.tile([128, TC, EA], F32, tag="ps", name=f"pv{i}{half}")
                # sc outer / t inner: each psum accumulation group must
                # be a contiguous matmul sequence within its bank (the
                # 4 sc-slices share one bank, so t-outer interleaving
                # corrupts accumulation — verified on HW)
                for sc in range(TC):
                    for t in range(TC):
                        nc.tensor.matmul(
                            pso_t[:, sc, :],
                            lhsT=pts[t][:, half, sc * 128 : (sc + 1) * 128],
                            rhs=vaugs[b][t][:, h, :],
                            start=(t == 0),
                            stop=(t == TC - 1),
                        )
                osb = opool.tile([128, TC, EA], FP16, tag="osb", name=f"o{i}_{half}")
                nc.vector.tensor_copy(
                    osb.rearrange("p a b -> p (a b)"),
                    pso_t.rearrange("p a b -> p (a b)"),
                )
                # last two items: both halves on sync so the end-of-NEFF
                # gpsimd drain has nothing in flight
                eng = nc.gpsimd if (half == 1 and i < N_ITEMS - 2) else nc.sync
                eng.dma_start(out=out_d[b, p, :, half], in_=osb)

            # ---- pipelined schedule. Both vprojs first (they only need
            # x + wv, which lead the DMA stream), then items with depth-4
            # lookahead: proj+scores run four items ahead of PV so exp
            # latency never gates the PE ----
            DEPTH = 4
            vproj(0)
            vproj(1)
            pending = {}
            for j in range(DEPTH):
                qt, kt, pts01 = proj_scores_a(j)
                pending[j] = pts01 + [st_pair(j, qt, kt, t) for t in range(2, TC)]
            for i in range(N_ITEMS):
                j = i + DEPTH
                if j < N_ITEMS:
                    qt, kt, pts01 = proj_scores_a(j)
                    pv_half(i, pending[i], 0)
                    pending[j] = pts01 + [st_pair(j, qt, kt, t) for t in range(2, TC)]
                else:
                    pv_half(i, pending[i], 0)
                pv_half(i, pending.pop(i), 1)

    legalize_waits(nc)
    return nc


def _prep_inputs(x, Wq, Wk, Wv):
    x = np.ascontiguousarray(np.asarray(x, dtype=np.float32))
    # x [B, S, D] -> per-core xT [B_LOC, C, 128, S]
    xt = x.reshape(N_CORES, B_LOC, S, D).transpose(0, 1, 3, 2)
    xt = np.ascontiguousarray(xt).reshape(N_CORES, B_LOC, C, 128, S).astype(np.float16)
    wp = []
    for W in (Wq, Wk, Wv):
        W = np.asarray(W, dtype=np.float32)
        # [H, D, DH] -> [D, H*DH] (d-major) -> [C, 128, H*DH]
        wp.append(
            np.ascontiguousarray(W.transpose(1, 0, 2))
            .reshape(C, 128, H * DH)
            .astype(np.float16)
        )
    return xt, wp[0], wp[1], wp[2]


_PROGRAM = None


def _get_program():
    global _PROGRAM
    if _PROGRAM is None:
        _PROGRAM = build_program()
    return _PROGRAM


def _finalize(raw):
    """raw: [B_LOC, HP, 128, 2, TC, EA] fp16 per core -> [B_LOC, S, D] fp32."""
    raw = raw.astype(np.float32)
    num = raw[..., :DH]  # [b, p, j, half, sc, e]
    den = raw[..., DH]  # [b, p, j, half, sc]
    o = num / den[..., None]
    # [b, p, j, half, sc, e] -> [b, sc, j, p, half, e] -> [b, s, d]
    return np.ascontiguousarray(o.transpose(0, 4, 2, 1, 3, 5)).reshape(B_LOC, S, D)


def run(x, Wq, Wk, Wv, trace=False, nc=None):
    xt, wq_p, wk_p, wv_p = _prep_inputs(x, Wq, Wk, Wv)
    if nc is None:
        nc = _get_program()
    in_maps = [
        {"xt": xt[i], "wq": wq_p, "wk": wk_p, "wv": wv_p} for i in range(N_CORES)
    ]
    res = run_bass_kernel_spmd(nc, in_maps, list(range(N_CORES)), trace=trace)
    out = np.concatenate(
        [_finalize(res.results[i]["out"]) for i in range(N_CORES)], axis=0
    )
    return out, res


def kernel(x, Wq, Wk, Wv):
    out, _ = run(x, Wq, Wk, Wv, trace=False)
    return out
